# revision 31
# baseline (speedup 1.0000x reference)
"""Trainium2 Bass kernel for nn_AttentionLayer (sparse graph attention + BatchNorm).

Strategy (8 NeuronCores, SPMD single NEFF):
  - Nodes are partitioned contiguously across cores (12500 per core); each edge is
    owned by the core that owns its dst node.
  - Per core, owned nodes are processed in windows of 128 (PSUM partition dim).
    Edges are grouped by dst window, padded to a uniform TPW tiles of 128 edges
    per window so one program serves all cores.
  - Per edge tile: gather x[src] rows via indirect DMA, transpose on PE, project
    K/V with resident weights; E projection uses host-pretransposed edge_attr
    tiles; Q is projected per-window from the contiguous x rows and broadcast to
    edges with a one-hot selection matmul; scores reduce per head, clip, exp;
    the segment-sum over dst is a one-hot matmul accumulated in PSUM per window.
  - BatchNorm: per-core partial sums reduced on host between two launches
    (reduce-then-broadcast), second tiny kernel applies the affine transform.
"""

import math
import numpy as np

import concourse.bass as bass
import concourse.tile as tile
from concourse import mybir
from concourse.bass import IndirectOffsetOnAxis
from concourse.bass_utils import run_bass_kernel_spmd

F32 = mybir.dt.float32
F32R = mybir.dt.float32r
I32 = mybir.dt.int32

CORES = 8
N = 100000
E = 500000
DIM = 256
H = 8
DH = DIM // H
NPC = N // CORES          # nodes per core
WIN = math.ceil(NPC / 128)  # windows of 128 nodes per core
EPS_Z = 1e-6
EPS_BN = 1e-5


# ----------------------------------------------------------------- host prep

def _prep(x, edge_attr, edge_index, cores, npc):
    """Partition edges by dst owner, group by dst window, pad to uniform TPW."""
    n = cores * npc
    win = math.ceil(npc / 128)
    src = np.asarray(edge_index[0], dtype=np.int64)
    dst = np.asarray(edge_index[1], dtype=np.int64)
    e = src.shape[0]
    x = np.asarray(x, dtype=np.float32)
    edge_attr = np.asarray(edge_attr, dtype=np.float32)

    owner = dst // npc
    ldst = dst - owner * npc
    wglob = owner * win + (ldst >> 7)
    slot = (ldst & 127).astype(np.int64)

    order = np.argsort(wglob, kind="stable")
    swin = wglob[order]
    cnt = np.bincount(wglob, minlength=cores * win)
    tpw = max(1, int(math.ceil(cnt.max() / 128)))
    cap = tpw * 128

    cum = np.zeros(cores * win, dtype=np.int64)
    cum[1:] = np.cumsum(cnt)[:-1]
    pos = np.arange(e, dtype=np.int64) - cum[swin]
    padded = np.full((cores * win, cap), -1, dtype=np.int64)
    padded[swin, pos] = order

    xrows = (cores - 1) * npc + win * 128
    xpad = np.zeros((xrows, DIM), dtype=np.float32)
    xpad[:n] = x

    t_per_core = win * tpw
    per_core = []
    for c in range(cores):
        esel = padded[c * win:(c + 1) * win].reshape(-1)  # [T*128]
        valid = esel >= 0
        esel_c = np.where(valid, esel, 0)

        idx_src = np.where(valid, src[esel_c], 0).astype(np.int32)
        idxcol = np.ascontiguousarray(idx_src.reshape(t_per_core, 128).T)

        slots = np.where(valid, slot[esel_c], -1).astype(np.float32)
        slot2d = slots.reshape(t_per_core, 128)
        slotcol = np.ascontiguousarray(slot2d.T)
        slotrow = np.ascontiguousarray(slot2d.reshape(win, cap))

        ea = edge_attr[esel_c] * valid[:, None].astype(np.float32)
        # [T,128e,256c] -> [T, 2ci, 128c, 128e]
        eat = np.ascontiguousarray(
            ea.reshape(t_per_core, 128, 2, 128).transpose(0, 2, 3, 1))

        per_core.append(dict(idxcol=idxcol, slotcol=slotcol,
                             slotrow=slotrow, eat=eat))
    consts = dict(
        iota_rowb=np.ascontiguousarray(
            np.tile(np.arange(128, dtype=np.float32), (128, 1))),
        iota_col=np.arange(128, dtype=np.float32).reshape(128, 1),
        ident=np.eye(128, dtype=np.float32),
    )
    return dict(xpad=xpad, per_core=per_core, consts=consts, tpw=tpw,
                cores=cores, npc=npc, win=win, xrows=xrows)


# ------------------------------------------------------------- phase1 kernel

def _build_phase1(npc, win, tpw, xrows, use_f32r=True, split_waits=True):
    from contextlib import ExitStack

    mdt = F32R if use_f32r else F32
    t_total = win * tpw
    real_last = npc - (win - 1) * 128  # real rows in the last window

    nc = bass.Bass()
    xpad_d = nc.declare_dram_parameter("xpad", [xrows, DIM], F32, isOutput=False)
    xwin_d = nc.declare_dram_parameter("xwin", [win * 128, DIM], F32, isOutput=False)
    eat_d = nc.declare_dram_parameter("eat", [t_total, 2, 128, 128], F32, isOutput=False)
    idxcol_d = nc.declare_dram_parameter("idxcol", [128, t_total], I32, isOutput=False)
    slotcol_d = nc.declare_dram_parameter("slotcol", [128, t_total], F32, isOutput=False)
    slotrow_d = nc.declare_dram_parameter("slotrow", [win, tpw * 128], F32, isOutput=False)
    w_d = {w: nc.declare_dram_parameter(w, [2, 128, DIM], F32, isOutput=False)
           for w in ("wq", "wk", "wv", "we")}
    iota_rowb_d = nc.declare_dram_parameter("iota_rowb", [128, 128], F32, isOutput=False)
    iota_col_d = nc.declare_dram_parameter("iota_col", [128, 1], F32, isOutput=False)
    ident_d = nc.declare_dram_parameter("ident", [128, 128], F32, isOutput=False)

    hpre_d = nc.declare_dram_parameter("hpre", [win * 128, DIM], F32, isOutput=True)
    bns_d = nc.declare_dram_parameter("bns", [2, DIM], F32, isOutput=True)

    mul = mybir.AluOpType.mult
    add = mybir.AluOpType.add
    iseq = mybir.AluOpType.is_equal

    with tile.TileContext(nc) as tc, ExitStack() as ctx:
        const = ctx.enter_context(tc.tile_pool(name="const", bufs=1))
        # resident weights [128, 2, 256]
        w_sb = {}
        for name, d in w_d.items():
            t = const.tile([128, 2, DIM], mdt, tag=f"w_{name}")
            nc.sync.dma_start(t[:], d[:].rearrange("c p f -> p c f").bitcast(mdt))
            w_sb[name] = t
        iota_rowb = const.tile([128, 128], F32, tag="iota_rowb")
        nc.sync.dma_start(iota_rowb[:], iota_rowb_d[:])
        iota_col = const.tile([128, 1], F32, tag="iota_col")
        nc.sync.dma_start(iota_col[:], iota_col_d[:])
        ident = const.tile([128, 128], F32, tag="ident")
        nc.sync.dma_start(ident[:], ident_d[:])
        idxcol = const.tile([128, t_total], I32, tag="idxcol")
        nc.sync.dma_start(idxcol[:], idxcol_d[:])
        slotcol = const.tile([128, t_total], F32, tag="slotcol")
        nc.sync.dma_start(slotcol[:], slotcol_d[:])
        ones_row = const.tile([1, 128], F32, tag="ones_row")
        nc.vector.memset(ones_row[:], 1.0)
        ones_col = const.tile([128, 1], F32, tag="ones_col")
        nc.vector.memset(ones_col[:], 1.0)
        bn1 = const.tile([128, DIM], F32, tag="bn1")
        nc.vector.memset(bn1[:], 0.0)
        bn2 = const.tile([128, DIM], F32, tag="bn2")
        nc.vector.memset(bn2[:], 0.0)

        # PSUM pools: pmm 4 banks, ptrans 3 banks, wvz 1 bank
        pmm = ctx.enter_context(tc.tile_pool(name="pmm", bufs=4, space="PSUM"))
        ptrans = ctx.enter_context(tc.tile_pool(name="ptrans", bufs=3, space="PSUM"))
        pwvz = ctx.enter_context(tc.tile_pool(name="pwvz", bufs=1, space="PSUM"))

        # SBUF pools
        p_xw = ctx.enter_context(tc.tile_pool(name="p_xw", bufs=3))
        p_xwt = ctx.enter_context(tc.tile_pool(name="p_xwt", bufs=2))
        p_ql = ctx.enter_context(tc.tile_pool(name="p_ql", bufs=2))
        p_srow = ctx.enter_context(tc.tile_pool(name="p_srow", bufs=2))
        p_xg = ctx.enter_context(tc.tile_pool(name="p_xg", bufs=2))
        p_ea = ctx.enter_context(tc.tile_pool(name="p_ea", bufs=2))
        p_xgt = ctx.enter_context(tc.tile_pool(name="p_xgt", bufs=3))
        p_sel = ctx.enter_context(tc.tile_pool(name="p_sel", bufs=3))
        p_m = ctx.enter_context(tc.tile_pool(name="p_m", bufs=3))
        p_small = ctx.enter_context(tc.tile_pool(name="p_small", bufs=4))
        p_h = ctx.enter_context(tc.tile_pool(name="p_h", bufs=2))

        def mm(out, lhsT, rhs, start, stop):
            nc.tensor.matmul(out, lhsT, rhs, start=start, stop=stop)

        for w in range(win):
            base = w * 128
            xw = p_xw.tile([128, DIM], F32, tag="xw")
            nc.scalar.dma_start(xw[:], xwin_d[base:base + 128, :])
            srow = p_srow.tile([1, tpw * 128], F32, tag="srow")
            nc.scalar.dma_start(srow[:], slotrow_d[w:w + 1, :])

            # transpose x window -> xwT [c(2ci*128), s]
            xwt_ps = ptrans.tile([128, 2, 128], F32, tag="ptrans")
            for ci in range(2):
                nc.tensor.transpose(xwt_ps[:, ci, :], xw[:, ci * 128:(ci + 1) * 128],
                                    ident[:])
            xwt = p_xwt.tile([128, 2, 128], mdt, tag="xwt")
            nc.scalar.copy(xwt[:], xwt_ps[:])

            # Q_local = x_w @ WQ  (WQ pre-scaled by 1/sqrt(DH) on host)
            ql_ps = ptrans.tile([128, DIM], F32, tag="ptrans")
            for ci in range(2):
                mm(ql_ps[:], xwt[:, ci, :], w_sb["wq"][:, ci, :], ci == 0, ci == 1)
            ql = p_ql.tile([128, DIM], mdt, tag="ql")
            nc.scalar.copy(ql[:], ql_ps[:])

            # gathers (one 2D-out indirect DMA per tile) + edge_attr load
            xg = p_xg.tile([128, tpw, DIM], F32, tag="xg")
            for t in range(tpw):
                nc.gpsimd.indirect_dma_start(
                    out=xg[:, t, :], out_offset=None, in_=xpad_d[:, :],
                    in_offset=IndirectOffsetOnAxis(
                        ap=idxcol[:, w * tpw + t:w * tpw + t + 1], axis=0))
            eaw = p_ea.tile([128, tpw, 2, 128], mdt, tag="eaw")
            nc.sync.dma_start(eaw[:],
                              eat_d[w * tpw:(w + 1) * tpw]
                              .rearrange("t c p e -> p t c e").bitcast(mdt))

            wvz = pwvz.tile([128, DIM + H], F32, tag="wvz")

            for t in range(tpw):
                tid = w * tpw + t
                xgt_ps = ptrans.tile([128, 2, 128], F32, tag="ptrans")
                for ci in range(2):
                    nc.tensor.transpose(xgt_ps[:, ci, :],
                                        xg[:, t, ci * 128:(ci + 1) * 128], ident[:])
                xgt = p_xgt.tile([128, 2, 128], mdt, tag="xgt")
                nc.scalar.copy(xgt[:], xgt_ps[:])

                kg = pmm.tile([128, DIM], F32, tag="pmm")
                for ci in range(2):
                    mm(kg[:], xgt[:, ci, :], w_sb["wk"][:, ci, :], ci == 0, ci == 1)
                vg = pmm.tile([128, DIM], F32, tag="pmm")
                for ci in range(2):
                    mm(vg[:], xgt[:, ci, :], w_sb["wv"][:, ci, :], ci == 0, ci == 1)
                eh = pmm.tile([128, DIM], F32, tag="pmm")
                for ci in range(2):
                    mm(eh[:], eaw[:, t, ci, :], w_sb["we"][:, ci, :], ci == 0, ci == 1)
                eh_sb = p_m.tile([128, DIM], F32, tag="ehsb")
                nc.scalar.copy(eh_sb[:], eh[:])

                # selection matrices
                sel = p_sel.tile([128, 128], mdt, tag="sel")
                nc.vector.tensor_scalar(
                    out=sel[:], in0=iota_rowb[:],
                    scalar1=slotcol[:, tid:tid + 1], scalar2=None, op0=iseq)
                dstb_ps = ptrans.tile([128, 128], F32, tag="ptrans")
                nc.tensor.matmul(dstb_ps[:], ones_row[:],
                                 srow[:, t * 128:(t + 1) * 128], start=True, stop=True)
                selt = p_sel.tile([128, 128], mdt, tag="selt")
                nc.vector.tensor_scalar(
                    out=selt[:], in0=dstb_ps[:], scalar1=iota_col[:],
                    scalar2=None, op0=iseq)

                # Qg = selt.T @ Q_local -> [e, f]
                qg = pmm.tile([128, DIM], F32, tag="pmm")
                mm(qg[:], selt[:], ql[:], True, True)

                # score pipeline
                m1 = p_m.tile([128, DIM], F32, tag="m1")
                nc.vector.tensor_tensor(out=m1[:], in0=kg[:], in1=eh_sb[:], op=mul)
                s2 = p_m.tile([128, DIM], F32, tag="s2")
                nc.vector.tensor_tensor(out=s2[:], in0=m1[:], in1=qg[:], op=mul)
                hs = p_small.tile([128, H], F32, tag="hs")
                nc.vector.tensor_reduce(
                    out=hs[:, :, None], in_=s2[:].rearrange("p (h d) -> p h d", d=DH),
                    op=add, axis=mybir.AxisListType.X)
                hc = p_small.tile([128, H], F32, tag="hc")
                nc.vector.tensor_scalar(out=hc[:], in0=hs[:], scalar1=5.0,
                                        scalar2=-5.0, op0=mybir.AluOpType.min,
                                        op1=mybir.AluOpType.max)
                msgz = p_m.tile([128, DIM + H], mdt, tag="msgz")
                nc.scalar.activation(msgz[:, DIM:DIM + H], hc[:],
                                     mybir.ActivationFunctionType.Exp)
                nc.vector.tensor_tensor(
                    out=msgz[:, 0:DIM].rearrange("p (h d) -> p h d", d=DH),
                    in0=vg[:].rearrange("p (h d) -> p h d", d=DH).bitcast(mdt),
                    in1=msgz[:, DIM:DIM + H, None].to_broadcast([128, H, DH]),
                    op=mul)

                # segment sum: wvz += sel.T @ [msg | score]
                mm(wvz[:], sel[:], msgz[:], t == 0, t == tpw - 1)

            # finalize window: h = x + wV / (Z + eps)
            zr = p_small.tile([128, H], F32, tag="zr")
            nc.vector.tensor_scalar(out=zr[:], in0=wvz[:, DIM:DIM + H],
                                    scalar1=EPS_Z, scalar2=None, op0=add)
            nc.vector.reciprocal(zr[:], zr[:])
            h = p_h.tile([128, DIM], F32, tag="h")
            nc.vector.tensor_tensor(
                out=h[:].rearrange("p (h d) -> p h d", d=DH),
                in0=wvz[:, 0:DIM].rearrange("p (h d) -> p h d", d=DH),
                in1=zr[:, :, None].to_broadcast([128, H, DH]), op=mul)
            nc.vector.tensor_tensor(out=h[:], in0=h[:], in1=xw[:], op=add)
            nc.scalar.dma_start(hpre_d[base:base + 128, :], h[:])

            sq = p_h.tile([128, DIM], F32, tag="sq")
            nc.scalar.square(sq[:], h[:])
            rows = real_last if w == win - 1 else 128
            nc.vector.tensor_tensor(out=bn1[:rows], in0=bn1[:rows],
                                    in1=h[:rows], op=add)
            nc.vector.tensor_tensor(out=bn2[:rows], in0=bn2[:rows],
                                    in1=sq[:rows], op=add)

        # cross-partition reduce of BN partials
        bn1_ps = pmm.tile([1, DIM], F32, tag="pmm")
        nc.tensor.matmul(bn1_ps[:], ones_col[:], bn1[:], start=True, stop=True)
        bn2_ps = pmm.tile([1, DIM], F32, tag="pmm")
        nc.tensor.matmul(bn2_ps[:], ones_col[:], bn2[:], start=True, stop=True)
        bn_sb = p_small.tile([1, 2 * DIM], F32, tag="bnsb")
        nc.vector.tensor_copy(bn_sb[:, 0:DIM], bn1_ps[:])
        nc.vector.tensor_copy(bn_sb[:, DIM:2 * DIM], bn2_ps[:])
        nc.scalar.dma_start(bns_d[:].rearrange("a b -> (a b)")[None, :], bn_sb[:])

    return _split_excess_waits(nc) if split_waits else nc


def _split_excess_waits(nc, max_waits=1):
    """Most HW-decoded opcodes carry only ~1 sync wait; move the excess onto
    preceding same-engine NoOps, which use the sequencer wait table."""
    k = 0
    skip = {"InstNoOp"}
    for f in nc.m.functions:
        for b in f.blocks:
            new = []
            for inst in b.instructions:
                si = inst.sync_info
                if (type(inst).__name__ not in skip and si is not None
                        and si.on_wait and len(si.on_wait) > max_waits):
                    extra = si.on_wait[:-max_waits]
                    for wt in extra:
                        nop = mybir.InstNoOp(name=f"I-wsplit{k}", ins=[], outs=[])
                        k += 1
                        nop.engine = inst.engine
                        nop.bass_nofuse = True
                        nop.sync_info = mybir.SyncInfo(on_wait=[wt], on_update=[])
                        new.append(nop)
                    inst.sync_info = mybir.SyncInfo(
                        on_wait=si.on_wait[-max_waits:], on_update=si.on_update)
                new.append(inst)
            b.instructions = new
    return nc


# ------------------------------------------------------------- phase2 kernel

def _build_phase2(win):
    from contextlib import ExitStack
    nc = bass.Bass()
    hpre_d = nc.declare_dram_parameter("hpre", [win * 128, DIM], F32, isOutput=False)
    scale_d = nc.declare_dram_parameter("scale_rep", [128, DIM], F32, isOutput=False)
    shift_d = nc.declare_dram_parameter("shift_rep", [128, DIM], F32, isOutput=False)
    hout_d = nc.declare_dram_parameter("hout", [win * 128, DIM], F32, isOutput=True)
    mul = mybir.AluOpType.mult
    add = mybir.AluOpType.add
    with tile.TileContext(nc) as tc, ExitStack() as ctx:
        const = ctx.enter_context(tc.tile_pool(name="const", bufs=1))
        scale = const.tile([128, DIM], F32, tag="scale")
        nc.sync.dma_start(scale[:], scale_d[:])
        shift = const.tile([128, DIM], F32, tag="shift")
        nc.sync.dma_start(shift[:], shift_d[:])
        pool = ctx.enter_context(tc.tile_pool(name="ht", bufs=4))
        for w in range(win):
            ht = pool.tile([128, DIM], F32, tag="ht")
            nc.sync.dma_start(ht[:], hpre_d[w * 128:(w + 1) * 128, :])
            nc.vector.tensor_tensor(out=ht[:], in0=ht[:], in1=scale[:], op=mul)
            nc.vector.tensor_tensor(out=ht[:], in0=ht[:], in1=shift[:], op=add)
            nc.scalar.dma_start(hout_d[w * 128:(w + 1) * 128, :], ht[:])
    return _split_excess_waits(nc)


# ------------------------------------------------------------------- runner

def _install_ntff_hook():
    """Install the antenv.axon_hooks shim so run_bass_kernel_spmd(trace=True)
    can capture NTFF profiles through the axon .so."""
    import sys, types
    if "antenv.axon_hooks" in sys.modules:
        return True
    try:
        import antenv
        from trn_agent_boot.trn_boot import _ntff_profile_via_ctypes
        mod = types.ModuleType("antenv.axon_hooks")
        mod._hook = _ntff_profile_via_ctypes("/opt/axon/libaxon_pjrt.so")
        mod.set_axon_ntff_profile_hook = lambda h: setattr(mod, "_hook", h)
        mod.get_axon_ntff_profile_hook = lambda: mod._hook
        sys.modules["antenv.axon_hooks"] = mod
        antenv.axon_hooks = mod
        return mod._hook is not None
    except Exception:
        return False


_CACHE = {}


def _get_phase1(npc, win, tpw, xrows, use_f32r):
    key = ("p1", npc, win, tpw, xrows, use_f32r)
    if key not in _CACHE:
        _CACHE[key] = _build_phase1(npc, win, tpw, xrows, use_f32r)
    return _CACHE[key]


def _get_phase2(win):
    key = ("p2", win)
    if key not in _CACHE:
        _CACHE[key] = _build_phase2(win)
    return _CACHE[key]


def run_pipeline(x, edge_attr, WQ, WK, WE, WV, gamma, beta, edge_index,
                 cores=CORES, npc=NPC, use_f32r=True, timed=False):
    n = cores * npc
    prep = _prep(x, edge_attr, edge_index, cores, npc)
    tpw, win, xrows = prep["tpw"], prep["win"], prep["xrows"]
    scale_inv = np.float32(1.0 / math.sqrt(DH))

    wq = np.ascontiguousarray((np.asarray(WQ, np.float32) * scale_inv)
                              .reshape(2, 128, DIM))
    wk = np.ascontiguousarray(np.asarray(WK, np.float32).reshape(2, 128, DIM))
    wv = np.ascontiguousarray(np.asarray(WV, np.float32).reshape(2, 128, DIM))
    we = np.ascontiguousarray(np.asarray(WE, np.float32).reshape(2, 128, DIM))

    nc1 = _get_phase1(npc, win, tpw, xrows, use_f32r)
    xpad = prep["xpad"]
    in_maps = []
    for c in range(cores):
        pc = prep["per_core"][c]
        xwin_c = np.ascontiguousarray(xpad[c * npc:c * npc + win * 128])
        in_maps.append(dict(
            xpad=xpad, xwin=xwin_c, eat=pc["eat"], idxcol=pc["idxcol"],
            slotcol=pc["slotcol"], slotrow=pc["slotrow"],
            wq=wq, wk=wk, wv=wv, we=we, **prep["consts"]))

    t1 = None
    trace = timed and _install_ntff_hook()
    r1 = run_bass_kernel_spmd(nc1, in_maps, list(range(cores)), trace=trace)
    results1 = r1.results
    t1 = r1.exec_time_ns
    hpres = [results1[c]["hpre"] for c in range(cores)]
    bns = sum(results1[c]["bns"].astype(np.float64) for c in range(cores))
    mean = bns[0] / n
    var = bns[1] / n - mean * mean
    scale = (np.asarray(gamma, np.float64) / np.sqrt(var + EPS_BN))
    shift = np.asarray(beta, np.float64) - mean * scale

    nc2 = _get_phase2(win)
    in_maps2 = [dict(hpre=hpres[c],
                     scale_rep=np.ascontiguousarray(
                         np.tile(scale.astype(np.float32), (128, 1))),
                     shift_rep=np.ascontiguousarray(
                         np.tile(shift.astype(np.float32), (128, 1))))
                for c in range(cores)]
    r2 = run_bass_kernel_spmd(nc2, in_maps2, list(range(cores)), trace=trace)
    results2 = r2.results
    t2 = r2.exec_time_ns
    out = np.concatenate([results2[c]["hout"][:npc] for c in range(cores)])
    info = dict(t1=t1, t2=t2, tpw=tpw)
    return out.astype(np.float32), info


def kernel(x, edge_attr, WQ, WK, WE, WV, gamma, beta, edge_index):
    out, _ = run_pipeline(x, edge_attr, WQ, WK, WE, WV, gamma, beta, edge_index)
    return out


# revision 40
# speedup vs baseline: 1.0705x; 1.0705x over previous
"""Trainium2 Bass kernel for nn_AttentionLayer (sparse graph attention + BatchNorm).

Strategy (8 NeuronCores, SPMD single NEFF):
  - Nodes are partitioned contiguously across cores (12500 per core); each edge is
    owned by the core that owns its dst node.
  - Per core, owned nodes are processed in windows of 128 (PSUM partition dim).
    Edges are grouped by dst window, padded to a uniform TPW tiles of 128 edges
    per window so one program serves all cores.
  - Per edge tile: gather x[src] rows via indirect DMA, transpose on PE, project
    K/V with resident weights; E projection uses host-pretransposed edge_attr
    tiles; Q is projected per-window from the contiguous x rows and broadcast to
    edges with a one-hot selection matmul; scores reduce per head, clip, exp;
    the segment-sum over dst is a one-hot matmul accumulated in PSUM per window.
  - BatchNorm: per-core partial sums reduced on host between two launches
    (reduce-then-broadcast), second tiny kernel applies the affine transform.
"""

import math
import numpy as np

import concourse.bass as bass
import concourse.tile as tile
from concourse import mybir
from concourse.bass import IndirectOffsetOnAxis
from concourse.bass_utils import run_bass_kernel_spmd

F32 = mybir.dt.float32
F32R = mybir.dt.float32r
I32 = mybir.dt.int32

CORES = 8
N = 100000
E = 500000
DIM = 256
H = 8
DH = DIM // H
NPC = N // CORES          # nodes per core
WIN = math.ceil(NPC / 128)  # windows of 128 nodes per core
EPS_Z = 1e-6
EPS_BN = 1e-5


# ----------------------------------------------------------------- host prep

def _prep(x, edge_attr, edge_index, cores, npc):
    """Partition edges by dst owner, group by dst window, pad to uniform TPW."""
    n = cores * npc
    win = math.ceil(npc / 128)
    src = np.asarray(edge_index[0], dtype=np.int64)
    dst = np.asarray(edge_index[1], dtype=np.int64)
    e = src.shape[0]
    x = np.asarray(x, dtype=np.float32)
    edge_attr = np.asarray(edge_attr, dtype=np.float32)

    owner = dst // npc
    ldst = dst - owner * npc
    wglob = owner * win + (ldst >> 7)
    slot = (ldst & 127).astype(np.int64)

    order = np.argsort(wglob, kind="stable")
    swin = wglob[order]
    cnt = np.bincount(wglob, minlength=cores * win)
    tpw = max(1, int(math.ceil(cnt.max() / 128)))
    cap = tpw * 128

    cum = np.zeros(cores * win, dtype=np.int64)
    cum[1:] = np.cumsum(cnt)[:-1]
    pos = np.arange(e, dtype=np.int64) - cum[swin]
    padded = np.full((cores * win, cap), -1, dtype=np.int64)
    padded[swin, pos] = order

    xrows = (cores - 1) * npc + win * 128
    xpad = np.zeros((xrows, DIM), dtype=np.float32)
    xpad[:n] = x

    t_per_core = win * tpw
    per_core = []
    for c in range(cores):
        esel = padded[c * win:(c + 1) * win].reshape(-1)  # [T*128]
        valid = esel >= 0
        esel_c = np.where(valid, esel, 0)

        idx_src = np.where(valid, src[esel_c], 0).astype(np.int32)
        idxcol = np.ascontiguousarray(idx_src.reshape(t_per_core, 128).T)

        slots = np.where(valid, slot[esel_c], -1).astype(np.float32)
        slot2d = slots.reshape(t_per_core, 128)
        slotcol = np.ascontiguousarray(slot2d.T)
        slotrow = np.ascontiguousarray(slot2d.reshape(win, cap))

        ea = edge_attr[esel_c] * valid[:, None].astype(np.float32)
        # [T,128e,256c] -> [T, 2ci, 128c, 128e]
        eat = np.ascontiguousarray(
            ea.reshape(t_per_core, 128, 2, 128).transpose(0, 2, 3, 1))

        xw_c = xpad[c * npc:c * npc + win * 128]
        # [win*128, 256] -> [2, 128, win*128] pretransposed lhsT chunks
        xwint = np.ascontiguousarray(
            xw_c.reshape(win * 128, 2, 128).transpose(1, 2, 0))
        per_core.append(dict(idxcol=idxcol, slotcol=slotcol,
                             slotrow=slotrow, eat=eat, xwint=xwint))
    consts = dict(
        iota_rowb=np.ascontiguousarray(
            np.tile(np.arange(128, dtype=np.float32), (128, 1))),
        iota_col=np.arange(128, dtype=np.float32).reshape(128, 1),
        ident=np.eye(128, dtype=np.float32),
        ones_row=np.ones((1, 128), dtype=np.float32),
    )
    return dict(xpad=xpad, per_core=per_core, consts=consts, tpw=tpw,
                cores=cores, npc=npc, win=win, xrows=xrows)


# ------------------------------------------------------------- phase1 kernel

def _build_phase1(npc, win, tpw, xrows, use_f32r=True, split_waits=True):
    from contextlib import ExitStack

    mdt = F32R if use_f32r else F32
    t_total = win * tpw
    real_last = npc - (win - 1) * 128  # real rows in the last window

    nc = bass.Bass()
    xpad_d = nc.declare_dram_parameter("xpad", [xrows, DIM], F32, isOutput=False)
    xwin_d = nc.declare_dram_parameter("xwin", [win * 128, DIM], F32, isOutput=False)
    eat_d = nc.declare_dram_parameter("eat", [t_total, 2, 128, 128], F32, isOutput=False)
    idxcol_d = nc.declare_dram_parameter("idxcol", [128, t_total], I32, isOutput=False)
    slotcol_d = nc.declare_dram_parameter("slotcol", [128, t_total], F32, isOutput=False)
    slotrow_d = nc.declare_dram_parameter("slotrow", [win, tpw * 128], F32, isOutput=False)
    wq_d = nc.declare_dram_parameter("wq", [2, 128, DIM], F32, isOutput=False)
    we_d = nc.declare_dram_parameter("we", [2, 128, DIM], F32, isOutput=False)
    wkv_d = nc.declare_dram_parameter("wkv", [2, 128, 2 * DIM], F32, isOutput=False)
    xwint_d = nc.declare_dram_parameter("xwint", [2, 128, win * 128], F32,
                                        isOutput=False)
    iota_rowb_d = nc.declare_dram_parameter("iota_rowb", [128, 128], F32, isOutput=False)
    iota_col_d = nc.declare_dram_parameter("iota_col", [128, 1], F32, isOutput=False)
    ident_d = nc.declare_dram_parameter("ident", [128, 128], F32, isOutput=False)
    ones_row_d = nc.declare_dram_parameter("ones_row", [1, 128], F32, isOutput=False)

    hpre_d = nc.declare_dram_parameter("hpre", [win * 128, DIM], F32, isOutput=True)
    bns_d = nc.declare_dram_parameter("bns", [2, DIM], F32, isOutput=True)

    mul = mybir.AluOpType.mult
    add = mybir.AluOpType.add
    iseq = mybir.AluOpType.is_equal

    with tile.TileContext(nc) as tc, ExitStack() as ctx:
        const = ctx.enter_context(tc.tile_pool(name="const", bufs=1))
        wq_sb = const.tile([128, 2, DIM], mdt, tag="wq")
        nc.sync.dma_start(wq_sb[:], wq_d[:].rearrange("c p f -> p c f").bitcast(mdt))
        we_sb = const.tile([128, 2, DIM], mdt, tag="we")
        nc.sync.dma_start(we_sb[:], we_d[:].rearrange("c p f -> p c f").bitcast(mdt))
        wkv_sb = const.tile([128, 2, 2 * DIM], mdt, tag="wkv")
        nc.sync.dma_start(wkv_sb[:],
                          wkv_d[:].rearrange("c p f -> p c f").bitcast(mdt))
        iota_rowb = const.tile([128, 128], F32, tag="iota_rowb")
        nc.sync.dma_start(iota_rowb[:], iota_rowb_d[:])
        iota_col = const.tile([128, 1], F32, tag="iota_col")
        nc.sync.dma_start(iota_col[:], iota_col_d[:])
        ident = const.tile([128, 128], mdt, tag="ident")
        nc.sync.dma_start(ident[:], ident_d[:].bitcast(mdt))
        idxcol = const.tile([128, t_total], I32, tag="idxcol")
        nc.sync.dma_start(idxcol[:], idxcol_d[:])
        slotcol = const.tile([128, t_total], F32, tag="slotcol")
        nc.sync.dma_start(slotcol[:], slotcol_d[:])
        ones_row = const.tile([1, 128], mdt, tag="ones_row")
        nc.sync.dma_start(ones_row[:], ones_row_d[:].bitcast(mdt))
        ones_col = const.tile([128, 1], F32, tag="ones_col")
        nc.vector.memset(ones_col[:], 1.0)
        bn1 = const.tile([128, DIM], F32, tag="bn1")
        nc.vector.memset(bn1[:], 0.0)
        bn2 = const.tile([128, DIM], F32, tag="bn2")
        nc.vector.memset(bn2[:], 0.0)

        # PSUM pools (8 banks): ptrans 2 (shared tag), kv 2, eh 1, qg 2, wvz 1
        ptrans = ctx.enter_context(tc.tile_pool(name="ptrans", bufs=2, space="PSUM"))
        pkv = ctx.enter_context(tc.tile_pool(name="pkv", bufs=2, space="PSUM"))
        peh = ctx.enter_context(tc.tile_pool(name="peh", bufs=1, space="PSUM"))
        pqg = ctx.enter_context(tc.tile_pool(name="pqg", bufs=2, space="PSUM"))
        pwvz = ctx.enter_context(tc.tile_pool(name="pwvz", bufs=1, space="PSUM"))

        # SBUF pools
        p_xw = ctx.enter_context(tc.tile_pool(name="p_xw", bufs=3))
        p_xwt = ctx.enter_context(tc.tile_pool(name="p_xwt", bufs=2))
        p_ql = ctx.enter_context(tc.tile_pool(name="p_ql", bufs=2))
        p_srow = ctx.enter_context(tc.tile_pool(name="p_srow", bufs=2))
        p_xg = ctx.enter_context(tc.tile_pool(name="p_xg", bufs=2))
        p_ea = ctx.enter_context(tc.tile_pool(name="p_ea", bufs=2))
        p_xgt = ctx.enter_context(tc.tile_pool(name="p_xgt", bufs=3))
        p_sel = ctx.enter_context(tc.tile_pool(name="p_sel", bufs=3))
        p_m = ctx.enter_context(tc.tile_pool(name="p_m", bufs=3))
        p_small = ctx.enter_context(tc.tile_pool(name="p_small", bufs=4))
        p_h = ctx.enter_context(tc.tile_pool(name="p_h", bufs=2))

        def mm(out, lhsT, rhs, start, stop):
            nc.tensor.matmul(out, lhsT, rhs, start=start, stop=stop)

        for w in range(win):
            base = w * 128
            xw = p_xw.tile([128, DIM], F32, tag="xw")
            nc.scalar.dma_start(xw[:], xwin_d[base:base + 128, :])
            srow = p_srow.tile([1, tpw * 128], mdt, tag="srow")
            nc.scalar.dma_start(srow[:], slotrow_d[w:w + 1, :].bitcast(mdt))

            # host-pretransposed x window chunks [c, s]
            xwt = p_xwt.tile([128, 2, 128], mdt, tag="xwt")
            nc.sync.dma_start(
                xwt[:], xwint_d[:, :, base:base + 128]
                .rearrange("c p s -> p c s").bitcast(mdt))

            # Q_local = x_w @ WQ  (WQ pre-scaled by 1/sqrt(DH) on host)
            ql_ps = ptrans.tile([128, DIM], F32, tag="ptrans")
            for ci in range(2):
                mm(ql_ps[:], xwt[:, ci, :], wq_sb[:, ci, :], ci == 0, ci == 1)
            ql = p_ql.tile([128, DIM], mdt, tag="ql")
            nc.scalar.copy(ql[:], ql_ps[:])

            # gathers (one 2D-out indirect DMA per tile) + edge_attr load
            xg = p_xg.tile([128, tpw, DIM], mdt, tag="xg")
            for t in range(tpw):
                nc.gpsimd.indirect_dma_start(
                    out=xg[:, t, :], out_offset=None, in_=xpad_d[:, :].bitcast(mdt),
                    in_offset=IndirectOffsetOnAxis(
                        ap=idxcol[:, w * tpw + t:w * tpw + t + 1], axis=0))
            eaw = p_ea.tile([128, tpw, 2, 128], mdt, tag="eaw")
            nc.sync.dma_start(eaw[:],
                              eat_d[w * tpw:(w + 1) * tpw]
                              .rearrange("t c p e -> p t c e").bitcast(mdt))

            wvz = pwvz.tile([128, DIM + H], F32, tag="wvz")

            dstb_ps = None
            for t in range(tpw):
                tid = w * tpw + t
                xgt_ps = ptrans.tile([128, 2, 128], F32, tag="ptrans")
                for ci in range(2):
                    nc.tensor.transpose(
                        xgt_ps[:, ci, :].bitcast(mdt),
                        xg[:, t, ci * 128:(ci + 1) * 128], ident[:])
                xgt = p_xgt.tile([128, 2, 128], mdt, tag="xgt")
                nc.scalar.copy(xgt[:], xgt_ps[:])

                kv = pkv.tile([128, 2 * DIM], F32, tag="kv")
                for ci in range(2):
                    mm(kv[:], xgt[:, ci, :], wkv_sb[:, ci, :], ci == 0, ci == 1)
                kg = kv[:, 0:DIM]
                vg = kv[:, DIM:2 * DIM]
                eh = peh.tile([128, DIM], F32, tag="eh")
                for ci in range(2):
                    mm(eh[:], eaw[:, t, ci, :], we_sb[:, ci, :], ci == 0, ci == 1)
                eh_sb = p_m.tile([128, DIM], F32, tag="ehsb")
                nc.scalar.copy(eh_sb[:], eh[:])

                # selection matrices; dstb built for pairs of tiles (N=256)
                sel = p_sel.tile([128, 128], mdt, tag="sel")
                nc.vector.tensor_scalar(
                    out=sel[:], in0=iota_rowb[:],
                    scalar1=slotcol[:, tid:tid + 1], scalar2=None, op0=iseq)
                if t % 2 == 0:
                    dstb_ps = ptrans.tile([128, 256], F32, tag="ptrans")
                    ncols = min(256, (tpw - t) * 128)
                    nc.tensor.matmul(
                        dstb_ps[:, :ncols], ones_row[:],
                        srow[:, t * 128:t * 128 + ncols],
                        start=True, stop=True)
                selt = p_sel.tile([128, 128], mdt, tag="selt")
                nc.vector.tensor_scalar(
                    out=selt[:], in0=dstb_ps[:, (t % 2) * 128:(t % 2 + 1) * 128],
                    scalar1=iota_col[:], scalar2=None, op0=iseq)

                # Qg = selt.T @ Q_local -> [e, f]
                qg = pqg.tile([128, DIM], F32, tag="qg")
                mm(qg[:], selt[:], ql[:], True, True)

                # score pipeline
                m1 = p_m.tile([128, DIM], F32, tag="m1")
                nc.vector.tensor_tensor(out=m1[:], in0=kg, in1=eh_sb[:], op=mul)
                s2 = p_m.tile([128, DIM], F32, tag="s2")
                nc.vector.tensor_tensor(out=s2[:], in0=m1[:], in1=qg[:], op=mul)
                hs = p_small.tile([128, H], F32, tag="hs")
                nc.vector.tensor_reduce(
                    out=hs[:, :, None], in_=s2[:].rearrange("p (h d) -> p h d", d=DH),
                    op=add, axis=mybir.AxisListType.X)
                hc = p_small.tile([128, H], F32, tag="hc")
                nc.vector.tensor_scalar(out=hc[:], in0=hs[:], scalar1=5.0,
                                        scalar2=-5.0, op0=mybir.AluOpType.min,
                                        op1=mybir.AluOpType.max)
                msgz = p_m.tile([128, DIM + H], mdt, tag="msgz")
                nc.scalar.activation(msgz[:, DIM:DIM + H], hc[:],
                                     mybir.ActivationFunctionType.Exp)
                nc.vector.tensor_tensor(
                    out=msgz[:, 0:DIM].rearrange("p (h d) -> p h d", d=DH),
                    in0=vg.rearrange("p (h d) -> p h d", d=DH).bitcast(mdt),
                    in1=msgz[:, DIM:DIM + H, None].to_broadcast([128, H, DH]),
                    op=mul)

                # segment sum: wvz += sel.T @ [msg | score]
                mm(wvz[:], sel[:], msgz[:], t == 0, t == tpw - 1)

            # finalize window: h = x + wV / (Z + eps)
            zr = p_small.tile([128, H], F32, tag="zr")
            nc.vector.tensor_scalar(out=zr[:], in0=wvz[:, DIM:DIM + H],
                                    scalar1=EPS_Z, scalar2=None, op0=add)
            nc.vector.reciprocal(zr[:], zr[:])
            h = p_h.tile([128, DIM], F32, tag="h")
            nc.vector.tensor_tensor(
                out=h[:].rearrange("p (h d) -> p h d", d=DH),
                in0=wvz[:, 0:DIM].rearrange("p (h d) -> p h d", d=DH),
                in1=zr[:, :, None].to_broadcast([128, H, DH]), op=mul)
            nc.gpsimd.tensor_tensor(out=h[:], in0=h[:], in1=xw[:], op=add)
            nc.scalar.dma_start(hpre_d[base:base + 128, :], h[:])

            sq = p_h.tile([128, DIM], F32, tag="sq")
            nc.scalar.square(sq[:], h[:])
            rows = real_last if w == win - 1 else 128
            nc.gpsimd.tensor_tensor(out=bn1[:rows], in0=bn1[:rows],
                                    in1=h[:rows], op=add)
            nc.gpsimd.tensor_tensor(out=bn2[:rows], in0=bn2[:rows],
                                    in1=sq[:rows], op=add)

        # cross-partition reduce of BN partials
        bn1_ps = peh.tile([1, DIM], F32, tag="eh")
        nc.tensor.matmul(bn1_ps[:], ones_col[:], bn1[:], start=True, stop=True)
        bn2_ps = pqg.tile([1, DIM], F32, tag="qg")
        nc.tensor.matmul(bn2_ps[:], ones_col[:], bn2[:], start=True, stop=True)
        bn_sb = p_small.tile([1, 2 * DIM], F32, tag="bnsb")
        nc.vector.tensor_copy(bn_sb[:, 0:DIM], bn1_ps[:])
        nc.vector.tensor_copy(bn_sb[:, DIM:2 * DIM], bn2_ps[:])
        nc.scalar.dma_start(bns_d[:].rearrange("a b -> (a b)")[None, :], bn_sb[:])

    return _split_excess_waits(nc) if split_waits else nc


def _split_excess_waits(nc, max_waits=1):
    """Most HW-decoded opcodes carry only ~1 sync wait; move the excess onto
    preceding same-engine NoOps, which use the sequencer wait table."""
    k = 0
    skip = {"InstNoOp"}
    for f in nc.m.functions:
        for b in f.blocks:
            new = []
            for inst in b.instructions:
                si = inst.sync_info
                if (type(inst).__name__ not in skip and si is not None
                        and si.on_wait and len(si.on_wait) > max_waits):
                    extra = si.on_wait[:-max_waits]
                    for wt in extra:
                        nop = mybir.InstNoOp(name=f"I-wsplit{k}", ins=[], outs=[])
                        k += 1
                        nop.engine = inst.engine
                        nop.bass_nofuse = True
                        nop.sync_info = mybir.SyncInfo(on_wait=[wt], on_update=[])
                        new.append(nop)
                    inst.sync_info = mybir.SyncInfo(
                        on_wait=si.on_wait[-max_waits:], on_update=si.on_update)
                new.append(inst)
            b.instructions = new
    return nc


# ------------------------------------------------------------- phase2 kernel

def _build_phase2(win):
    from contextlib import ExitStack
    nc = bass.Bass()
    hpre_d = nc.declare_dram_parameter("hpre", [win * 128, DIM], F32, isOutput=False)
    scale_d = nc.declare_dram_parameter("scale_rep", [128, DIM], F32, isOutput=False)
    shift_d = nc.declare_dram_parameter("shift_rep", [128, DIM], F32, isOutput=False)
    hout_d = nc.declare_dram_parameter("hout", [win * 128, DIM], F32, isOutput=True)
    mul = mybir.AluOpType.mult
    add = mybir.AluOpType.add
    with tile.TileContext(nc) as tc, ExitStack() as ctx:
        const = ctx.enter_context(tc.tile_pool(name="const", bufs=1))
        scale = const.tile([128, DIM], F32, tag="scale")
        nc.sync.dma_start(scale[:], scale_d[:])
        shift = const.tile([128, DIM], F32, tag="shift")
        nc.sync.dma_start(shift[:], shift_d[:])
        pool = ctx.enter_context(tc.tile_pool(name="ht", bufs=4))
        hp = hpre_d[:].rearrange("(b w p) f -> b p w f", p=128, w=2)
        ho = hout_d[:].rearrange("(b w p) f -> b p w f", p=128, w=2)
        nblk = hp.shape[0]
        for b in range(nblk):
            ht = pool.tile([128, 2, DIM], F32, tag="ht")
            nc.sync.dma_start(ht[:], hp[b])
            nc.vector.tensor_tensor(
                out=ht[:], in0=ht[:],
                in1=scale[:, None, :].to_broadcast([128, 2, DIM]), op=mul)
            nc.vector.tensor_tensor(
                out=ht[:], in0=ht[:],
                in1=shift[:, None, :].to_broadcast([128, 2, DIM]), op=add)
            nc.scalar.dma_start(ho[b], ht[:])
    return _split_excess_waits(nc)


# ------------------------------------------------------------------- runner

def _install_ntff_hook():
    """Install the antenv.axon_hooks shim so run_bass_kernel_spmd(trace=True)
    can capture NTFF profiles through the axon .so."""
    import sys, types
    if "antenv.axon_hooks" in sys.modules:
        return True
    try:
        import antenv
        from trn_agent_boot.trn_boot import _ntff_profile_via_ctypes
        mod = types.ModuleType("antenv.axon_hooks")
        mod._hook = _ntff_profile_via_ctypes("/opt/axon/libaxon_pjrt.so")
        mod.set_axon_ntff_profile_hook = lambda h: setattr(mod, "_hook", h)
        mod.get_axon_ntff_profile_hook = lambda: mod._hook
        sys.modules["antenv.axon_hooks"] = mod
        antenv.axon_hooks = mod
        return mod._hook is not None
    except Exception:
        return False


_CACHE = {}


def _get_phase1(npc, win, tpw, xrows, use_f32r):
    key = ("p1", npc, win, tpw, xrows, use_f32r)
    if key not in _CACHE:
        _CACHE[key] = _build_phase1(npc, win, tpw, xrows, use_f32r)
    return _CACHE[key]


def _get_phase2(win):
    key = ("p2", win)
    if key not in _CACHE:
        _CACHE[key] = _build_phase2(win)
    return _CACHE[key]


def run_pipeline(x, edge_attr, WQ, WK, WE, WV, gamma, beta, edge_index,
                 cores=CORES, npc=NPC, use_f32r=True, timed=False):
    n = cores * npc
    prep = _prep(x, edge_attr, edge_index, cores, npc)
    tpw, win, xrows = prep["tpw"], prep["win"], prep["xrows"]
    scale_inv = np.float32(1.0 / math.sqrt(DH))

    wq = np.ascontiguousarray((np.asarray(WQ, np.float32) * scale_inv)
                              .reshape(2, 128, DIM))
    we = np.ascontiguousarray(np.asarray(WE, np.float32).reshape(2, 128, DIM))
    wkv = np.ascontiguousarray(np.concatenate(
        [np.asarray(WK, np.float32).reshape(2, 128, DIM),
         np.asarray(WV, np.float32).reshape(2, 128, DIM)], axis=2))

    nc1 = _get_phase1(npc, win, tpw, xrows, use_f32r)
    xpad = prep["xpad"]
    in_maps = []
    for c in range(cores):
        pc = prep["per_core"][c]
        xwin_c = np.ascontiguousarray(xpad[c * npc:c * npc + win * 128])
        in_maps.append(dict(
            xpad=xpad, xwin=xwin_c, eat=pc["eat"], idxcol=pc["idxcol"],
            slotcol=pc["slotcol"], slotrow=pc["slotrow"], xwint=pc["xwint"],
            wq=wq, we=we, wkv=wkv, **prep["consts"]))

    t1 = None
    trace = timed and _install_ntff_hook()
    r1 = run_bass_kernel_spmd(nc1, in_maps, list(range(cores)), trace=trace)
    results1 = r1.results
    t1 = r1.exec_time_ns
    hpres = [results1[c]["hpre"] for c in range(cores)]
    bns = sum(results1[c]["bns"].astype(np.float64) for c in range(cores))
    mean = bns[0] / n
    var = bns[1] / n - mean * mean
    scale = (np.asarray(gamma, np.float64) / np.sqrt(var + EPS_BN))
    shift = np.asarray(beta, np.float64) - mean * scale

    nc2 = _get_phase2(win)
    in_maps2 = [dict(hpre=hpres[c],
                     scale_rep=np.ascontiguousarray(
                         np.tile(scale.astype(np.float32), (128, 1))),
                     shift_rep=np.ascontiguousarray(
                         np.tile(shift.astype(np.float32), (128, 1))))
                for c in range(cores)]
    r2 = run_bass_kernel_spmd(nc2, in_maps2, list(range(cores)), trace=trace)
    results2 = r2.results
    t2 = r2.exec_time_ns
    out = np.concatenate([results2[c]["hout"][:npc] for c in range(cores)])
    info = dict(t1=t1, t2=t2, tpw=tpw)
    return out.astype(np.float32), info


def kernel(x, edge_attr, WQ, WK, WE, WV, gamma, beta, edge_index):
    out, _ = run_pipeline(x, edge_attr, WQ, WK, WE, WV, gamma, beta, edge_index)
    return out


# revision 44
# speedup vs baseline: 1.0790x; 1.0079x over previous
"""Trainium2 Bass kernel for nn_AttentionLayer (sparse graph attention + BatchNorm).

Strategy (8 NeuronCores, SPMD single NEFF):
  - Nodes are partitioned contiguously across cores (12500 per core); each edge is
    owned by the core that owns its dst node.
  - Per core, owned nodes are processed in windows of 128 (PSUM partition dim).
    Edges are grouped by dst window, padded to a uniform TPW tiles of 128 edges
    per window so one program serves all cores.
  - Per edge tile: gather x[src] rows via indirect DMA, transpose on PE, project
    K/V with resident weights; E projection uses host-pretransposed edge_attr
    tiles; Q is projected per-window from the contiguous x rows and broadcast to
    edges with a one-hot selection matmul; scores reduce per head, clip, exp;
    the segment-sum over dst is a one-hot matmul accumulated in PSUM per window.
  - BatchNorm: per-core partial sums reduced on host between two launches
    (reduce-then-broadcast), second tiny kernel applies the affine transform.
"""

import math
import numpy as np

import concourse.bass as bass
import concourse.tile as tile
from concourse import mybir
from concourse.bass import IndirectOffsetOnAxis
from concourse.bass_utils import run_bass_kernel_spmd

F32 = mybir.dt.float32
F32R = mybir.dt.float32r
BF16 = mybir.dt.bfloat16
I32 = mybir.dt.int32

CORES = 8
N = 100000
E = 500000
DIM = 256
H = 8
DH = DIM // H
NPC = N // CORES          # nodes per core
WIN = math.ceil(NPC / 128)  # windows of 128 nodes per core
EPS_Z = 1e-6
EPS_BN = 1e-5


# ----------------------------------------------------------------- host prep

def _prep(x, edge_attr, edge_index, cores, npc):
    """Partition edges by dst owner, group by dst window, pad to uniform TPW."""
    n = cores * npc
    win = math.ceil(npc / 128)
    src = np.asarray(edge_index[0], dtype=np.int64)
    dst = np.asarray(edge_index[1], dtype=np.int64)
    e = src.shape[0]
    x = np.asarray(x, dtype=np.float32)
    edge_attr = np.asarray(edge_attr, dtype=np.float32)

    owner = dst // npc
    ldst = dst - owner * npc
    wglob = owner * win + (ldst >> 7)
    slot = (ldst & 127).astype(np.int64)

    order = np.argsort(wglob, kind="stable")
    swin = wglob[order]
    cnt = np.bincount(wglob, minlength=cores * win)
    tpw = max(1, int(math.ceil(cnt.max() / 128)))
    cap = tpw * 128

    cum = np.zeros(cores * win, dtype=np.int64)
    cum[1:] = np.cumsum(cnt)[:-1]
    pos = np.arange(e, dtype=np.int64) - cum[swin]
    padded = np.full((cores * win, cap), -1, dtype=np.int64)
    padded[swin, pos] = order

    xrows = (cores - 1) * npc + win * 128
    xpad = np.zeros((xrows, DIM), dtype=np.float32)
    xpad[:n] = x

    t_per_core = win * tpw
    per_core = []
    for c in range(cores):
        esel = padded[c * win:(c + 1) * win].reshape(-1)  # [T*128]
        valid = esel >= 0
        esel_c = np.where(valid, esel, 0)

        idx_src = np.where(valid, src[esel_c], 0).astype(np.int32)
        idxcol = np.ascontiguousarray(idx_src.reshape(t_per_core, 128).T)

        slots = np.where(valid, slot[esel_c], -1).astype(np.float32)
        slot2d = slots.reshape(t_per_core, 128)
        slotcol = np.ascontiguousarray(slot2d.T)
        slotrow = np.ascontiguousarray(slot2d.reshape(win, cap))

        ea = edge_attr[esel_c] * valid[:, None].astype(np.float32)
        # [T,128e,256c] -> [T, 2ci, 128c, 128e]
        eat = np.ascontiguousarray(
            ea.reshape(t_per_core, 128, 2, 128).transpose(0, 2, 3, 1))

        xw_c = xpad[c * npc:c * npc + win * 128]
        # [win*128, 256] -> [2, 128, win*128] pretransposed lhsT chunks
        xwint = np.ascontiguousarray(
            xw_c.reshape(win * 128, 2, 128).transpose(1, 2, 0))
        per_core.append(dict(idxcol=idxcol, slotcol=slotcol,
                             slotrow=slotrow, eat=eat, xwint=xwint))
    consts = dict(
        iota_rowb=np.ascontiguousarray(
            np.tile(np.arange(128, dtype=np.float32), (128, 1))),
        iota_col=np.arange(128, dtype=np.float32).reshape(128, 1),
        ident=np.eye(128, dtype=np.float32),
        ones_row=np.ones((1, 128), dtype=np.float32),
    )
    return dict(xpad=xpad, per_core=per_core, consts=consts, tpw=tpw,
                cores=cores, npc=npc, win=win, xrows=xrows)


# ------------------------------------------------------------- phase1 kernel

def _build_phase1(npc, win, tpw, xrows, prec="f32r", split_waits=True):
    from contextlib import ExitStack

    mdt = {"f32r": F32R, "f32": F32, "bf16": BF16}[prec]
    t_total = win * tpw
    real_last = npc - (win - 1) * 128  # real rows in the last window

    nc = bass.Bass()
    xtab_d = nc.declare_dram_parameter("xtab", [xrows, DIM], mdt, isOutput=False)
    xwin_d = nc.declare_dram_parameter("xwin", [win * 128, DIM], F32, isOutput=False)
    eat_d = nc.declare_dram_parameter("eat", [t_total, 2, 128, 128], mdt, isOutput=False)
    idxcol_d = nc.declare_dram_parameter("idxcol", [128, t_total], I32, isOutput=False)
    slotcol_d = nc.declare_dram_parameter("slotcol", [128, t_total], F32, isOutput=False)
    slotrow_d = nc.declare_dram_parameter("slotrow", [win, tpw * 128], mdt, isOutput=False)
    wq_d = nc.declare_dram_parameter("wq", [2, 128, DIM], mdt, isOutput=False)
    we_d = nc.declare_dram_parameter("we", [2, 128, DIM], mdt, isOutput=False)
    wkv_d = nc.declare_dram_parameter("wkv", [2, 128, 2 * DIM], mdt, isOutput=False)
    xwint_d = nc.declare_dram_parameter("xwint", [2, 128, win * 128], mdt,
                                        isOutput=False)
    iota_rowb_d = nc.declare_dram_parameter("iota_rowb", [128, 128], mdt, isOutput=False)
    iota_col_d = nc.declare_dram_parameter("iota_col", [128, 1], F32, isOutput=False)
    ident_d = nc.declare_dram_parameter("ident", [128, 128], mdt, isOutput=False)
    ones_row_d = nc.declare_dram_parameter("ones_row", [1, 128], mdt, isOutput=False)

    hpre_d = nc.declare_dram_parameter("hpre", [win * 128, DIM], F32, isOutput=True)
    bns_d = nc.declare_dram_parameter("bns", [2, DIM], F32, isOutput=True)

    mul = mybir.AluOpType.mult
    add = mybir.AluOpType.add
    iseq = mybir.AluOpType.is_equal

    with tile.TileContext(nc) as tc, ExitStack() as ctx:
        const = ctx.enter_context(tc.tile_pool(name="const", bufs=1))
        wq_sb = const.tile([128, 2, DIM], mdt, tag="wq")
        nc.sync.dma_start(wq_sb[:], wq_d[:].rearrange("c p f -> p c f"))
        we_sb = const.tile([128, 2, DIM], mdt, tag="we")
        nc.sync.dma_start(we_sb[:], we_d[:].rearrange("c p f -> p c f"))
        wkv_sb = const.tile([128, 2, 2 * DIM], mdt, tag="wkv")
        nc.sync.dma_start(wkv_sb[:],
                          wkv_d[:].rearrange("c p f -> p c f"))
        iota_rowb = const.tile([128, 128], mdt, tag="iota_rowb")
        nc.sync.dma_start(iota_rowb[:], iota_rowb_d[:])
        iota_col = const.tile([128, 1], F32, tag="iota_col")
        nc.sync.dma_start(iota_col[:], iota_col_d[:])
        ident = const.tile([128, 128], mdt, tag="ident")
        nc.sync.dma_start(ident[:], ident_d[:])
        idxcol = const.tile([128, t_total], I32, tag="idxcol")
        nc.sync.dma_start(idxcol[:], idxcol_d[:])
        slotcol = const.tile([128, t_total], F32, tag="slotcol")
        nc.sync.dma_start(slotcol[:], slotcol_d[:])
        ones_row = const.tile([1, 128], mdt, tag="ones_row")
        nc.sync.dma_start(ones_row[:], ones_row_d[:])
        ones_col = const.tile([128, 1], F32, tag="ones_col")
        nc.vector.memset(ones_col[:], 1.0)
        bn1 = const.tile([128, DIM], F32, tag="bn1")
        nc.vector.memset(bn1[:], 0.0)
        bn2 = const.tile([128, DIM], F32, tag="bn2")
        nc.vector.memset(bn2[:], 0.0)

        # PSUM pools (8 banks): ptrans 2 (shared tag), kv 2, eh 1, qg 2, wvz 1
        ptrans = ctx.enter_context(tc.tile_pool(name="ptrans", bufs=2, space="PSUM"))
        pkv = ctx.enter_context(tc.tile_pool(name="pkv", bufs=2, space="PSUM"))
        peh = ctx.enter_context(tc.tile_pool(name="peh", bufs=1, space="PSUM"))
        pqg = ctx.enter_context(tc.tile_pool(name="pqg", bufs=2, space="PSUM"))
        pwvz = ctx.enter_context(tc.tile_pool(name="pwvz", bufs=1, space="PSUM"))

        # SBUF pools
        p_xw = ctx.enter_context(tc.tile_pool(name="p_xw", bufs=3))
        p_xwt = ctx.enter_context(tc.tile_pool(name="p_xwt", bufs=2))
        p_ql = ctx.enter_context(tc.tile_pool(name="p_ql", bufs=2))
        p_srow = ctx.enter_context(tc.tile_pool(name="p_srow", bufs=2))
        p_xg = ctx.enter_context(tc.tile_pool(name="p_xg", bufs=2))
        p_ea = ctx.enter_context(tc.tile_pool(name="p_ea", bufs=2))
        p_xgt = ctx.enter_context(tc.tile_pool(name="p_xgt", bufs=3))
        p_sel = ctx.enter_context(tc.tile_pool(name="p_sel", bufs=3))
        p_m = ctx.enter_context(tc.tile_pool(name="p_m", bufs=3))
        p_small = ctx.enter_context(tc.tile_pool(name="p_small", bufs=4))
        p_h = ctx.enter_context(tc.tile_pool(name="p_h", bufs=2))

        def mm(out, lhsT, rhs, start, stop):
            nc.tensor.matmul(out, lhsT, rhs, start=start, stop=stop)

        for w in range(win):
            base = w * 128
            xw = p_xw.tile([128, DIM], F32, tag="xw")
            nc.scalar.dma_start(xw[:], xwin_d[base:base + 128, :])
            srow = p_srow.tile([1, tpw * 128], mdt, tag="srow")
            nc.scalar.dma_start(srow[:], slotrow_d[w:w + 1, :])

            # host-pretransposed x window chunks [c, s]
            xwt = p_xwt.tile([128, 2, 128], mdt, tag="xwt")
            nc.sync.dma_start(
                xwt[:], xwint_d[:, :, base:base + 128]
                .rearrange("c p s -> p c s"))

            # Q_local = x_w @ WQ  (WQ pre-scaled by 1/sqrt(DH) on host)
            ql_ps = ptrans.tile([128, DIM], F32, tag="ptrans")
            for ci in range(2):
                mm(ql_ps[:], xwt[:, ci, :], wq_sb[:, ci, :], ci == 0, ci == 1)
            ql = p_ql.tile([128, DIM], mdt, tag="ql")
            nc.scalar.copy(ql[:], ql_ps[:])

            # gathers (one 2D-out indirect DMA per tile) + edge_attr load
            xg = p_xg.tile([128, tpw, DIM], mdt, tag="xg")
            for t in range(tpw):
                nc.gpsimd.indirect_dma_start(
                    out=xg[:, t, :], out_offset=None, in_=xtab_d[:, :],
                    in_offset=IndirectOffsetOnAxis(
                        ap=idxcol[:, w * tpw + t:w * tpw + t + 1], axis=0))
            eaw = p_ea.tile([128, tpw, 2, 128], mdt, tag="eaw")
            nc.sync.dma_start(eaw[:],
                              eat_d[w * tpw:(w + 1) * tpw]
                              .rearrange("t c p e -> p t c e"))

            wvz = pwvz.tile([128, DIM + H], F32, tag="wvz")

            dstb_ps = None
            for t in range(tpw):
                tid = w * tpw + t
                xgt = p_xgt.tile([128, 2, 128], mdt, tag="xgt")
                if mdt == BF16:
                    # xbar DMA transpose, SBUF->SBUF, per 128x128 chunk
                    for ci in range(2):
                        nc.sync.dma_start(
                            out=xgt[:, ci, :],
                            in_=xg[:, t, ci * 128:(ci + 1) * 128],
                            transpose=True)
                else:
                    xgt_ps = ptrans.tile([128, 2, 128], F32, tag="ptrans")
                    for ci in range(2):
                        nc.tensor.transpose(
                            xgt_ps[:, ci, :].bitcast(mdt),
                            xg[:, t, ci * 128:(ci + 1) * 128], ident[:])
                    nc.scalar.copy(xgt[:], xgt_ps[:])

                kv = pkv.tile([128, 2 * DIM], F32, tag="kv")
                for ci in range(2):
                    mm(kv[:], xgt[:, ci, :], wkv_sb[:, ci, :], ci == 0, ci == 1)
                kg = kv[:, 0:DIM]
                vg = kv[:, DIM:2 * DIM]
                eh = peh.tile([128, DIM], F32, tag="eh")
                for ci in range(2):
                    mm(eh[:], eaw[:, t, ci, :], we_sb[:, ci, :], ci == 0, ci == 1)
                eh_sb = p_m.tile([128, DIM], F32, tag="ehsb")
                nc.scalar.copy(eh_sb[:], eh[:])

                # selection matrices; dstb built for pairs of tiles (N=256)
                sel = p_sel.tile([128, 128], mdt, tag="sel")
                nc.vector.tensor_scalar(
                    out=sel[:], in0=iota_rowb[:],
                    scalar1=slotcol[:, tid:tid + 1], scalar2=None, op0=iseq)
                if t % 2 == 0:
                    dstb_ps = ptrans.tile([128, 256], F32, tag="ptrans")
                    ncols = min(256, (tpw - t) * 128)
                    nc.tensor.matmul(
                        dstb_ps[:, :ncols], ones_row[:],
                        srow[:, t * 128:t * 128 + ncols],
                        start=True, stop=True)
                selt = p_sel.tile([128, 128], mdt, tag="selt")
                nc.vector.tensor_scalar(
                    out=selt[:], in0=dstb_ps[:, (t % 2) * 128:(t % 2 + 1) * 128],
                    scalar1=iota_col[:], scalar2=None, op0=iseq)

                # Qg = selt.T @ Q_local -> [e, f]
                qg = pqg.tile([128, DIM], F32, tag="qg")
                mm(qg[:], selt[:], ql[:], True, True)

                # score pipeline
                m1 = p_m.tile([128, DIM], F32, tag="m1")
                nc.vector.tensor_tensor(out=m1[:], in0=kg, in1=eh_sb[:], op=mul)
                s2 = p_m.tile([128, DIM], F32, tag="s2")
                nc.vector.tensor_tensor(out=s2[:], in0=m1[:], in1=qg[:], op=mul)
                hs = p_small.tile([128, H], F32, tag="hs")
                nc.vector.tensor_reduce(
                    out=hs[:, :, None], in_=s2[:].rearrange("p (h d) -> p h d", d=DH),
                    op=add, axis=mybir.AxisListType.X)
                hc = p_small.tile([128, H], F32, tag="hc")
                nc.vector.tensor_scalar(out=hc[:], in0=hs[:], scalar1=5.0,
                                        scalar2=-5.0, op0=mybir.AluOpType.min,
                                        op1=mybir.AluOpType.max)
                msgz = p_m.tile([128, DIM + H], mdt, tag="msgz")
                nc.scalar.activation(msgz[:, DIM:DIM + H], hc[:],
                                     mybir.ActivationFunctionType.Exp)
                nc.vector.tensor_tensor(
                    out=msgz[:, 0:DIM].rearrange("p (h d) -> p h d", d=DH),
                    in0=vg.rearrange("p (h d) -> p h d", d=DH),
                    in1=msgz[:, DIM:DIM + H, None].to_broadcast([128, H, DH]),
                    op=mul)

                # segment sum: wvz += sel.T @ [msg | score]
                mm(wvz[:], sel[:], msgz[:], t == 0, t == tpw - 1)

            # finalize window: h = x + wV / (Z + eps)
            zr = p_small.tile([128, H], F32, tag="zr")
            nc.vector.tensor_scalar(out=zr[:], in0=wvz[:, DIM:DIM + H],
                                    scalar1=EPS_Z, scalar2=None, op0=add)
            nc.vector.reciprocal(zr[:], zr[:])
            h = p_h.tile([128, DIM], F32, tag="h")
            nc.vector.tensor_tensor(
                out=h[:].rearrange("p (h d) -> p h d", d=DH),
                in0=wvz[:, 0:DIM].rearrange("p (h d) -> p h d", d=DH),
                in1=zr[:, :, None].to_broadcast([128, H, DH]), op=mul)
            nc.gpsimd.tensor_tensor(out=h[:], in0=h[:], in1=xw[:], op=add)
            nc.scalar.dma_start(hpre_d[base:base + 128, :], h[:])

            sq = p_h.tile([128, DIM], F32, tag="sq")
            nc.scalar.square(sq[:], h[:])
            rows = real_last if w == win - 1 else 128
            nc.gpsimd.tensor_tensor(out=bn1[:rows], in0=bn1[:rows],
                                    in1=h[:rows], op=add)
            nc.gpsimd.tensor_tensor(out=bn2[:rows], in0=bn2[:rows],
                                    in1=sq[:rows], op=add)

        # cross-partition reduce of BN partials
        bn1_ps = peh.tile([1, DIM], F32, tag="eh")
        nc.tensor.matmul(bn1_ps[:], ones_col[:], bn1[:], start=True, stop=True)
        bn2_ps = pqg.tile([1, DIM], F32, tag="qg")
        nc.tensor.matmul(bn2_ps[:], ones_col[:], bn2[:], start=True, stop=True)
        bn_sb = p_small.tile([1, 2 * DIM], F32, tag="bnsb")
        nc.vector.tensor_copy(bn_sb[:, 0:DIM], bn1_ps[:])
        nc.vector.tensor_copy(bn_sb[:, DIM:2 * DIM], bn2_ps[:])
        nc.scalar.dma_start(bns_d[:].rearrange("a b -> (a b)")[None, :], bn_sb[:])

    return _split_excess_waits(nc) if split_waits else nc


def _split_excess_waits(nc, max_waits=1):
    """Most HW-decoded opcodes carry only ~1 sync wait; move the excess onto
    preceding same-engine NoOps, which use the sequencer wait table."""
    k = 0
    skip = {"InstNoOp"}
    for f in nc.m.functions:
        for b in f.blocks:
            new = []
            for inst in b.instructions:
                si = inst.sync_info
                if (type(inst).__name__ not in skip and si is not None
                        and si.on_wait and len(si.on_wait) > max_waits):
                    extra = si.on_wait[:-max_waits]
                    for wt in extra:
                        nop = mybir.InstNoOp(name=f"I-wsplit{k}", ins=[], outs=[])
                        k += 1
                        nop.engine = inst.engine
                        nop.bass_nofuse = True
                        nop.sync_info = mybir.SyncInfo(on_wait=[wt], on_update=[])
                        new.append(nop)
                    inst.sync_info = mybir.SyncInfo(
                        on_wait=si.on_wait[-max_waits:], on_update=si.on_update)
                new.append(inst)
            b.instructions = new
    return nc


# ------------------------------------------------------------- phase2 kernel

def _build_phase2(win):
    from contextlib import ExitStack
    nc = bass.Bass()
    hpre_d = nc.declare_dram_parameter("hpre", [win * 128, DIM], F32, isOutput=False)
    scale_d = nc.declare_dram_parameter("scale_rep", [128, DIM], F32, isOutput=False)
    shift_d = nc.declare_dram_parameter("shift_rep", [128, DIM], F32, isOutput=False)
    hout_d = nc.declare_dram_parameter("hout", [win * 128, DIM], F32, isOutput=True)
    mul = mybir.AluOpType.mult
    add = mybir.AluOpType.add
    with tile.TileContext(nc) as tc, ExitStack() as ctx:
        const = ctx.enter_context(tc.tile_pool(name="const", bufs=1))
        scale = const.tile([128, DIM], F32, tag="scale")
        nc.sync.dma_start(scale[:], scale_d[:])
        shift = const.tile([128, DIM], F32, tag="shift")
        nc.sync.dma_start(shift[:], shift_d[:])
        pool = ctx.enter_context(tc.tile_pool(name="ht", bufs=4))
        hp = hpre_d[:].rearrange("(b w p) f -> b p w f", p=128, w=2)
        ho = hout_d[:].rearrange("(b w p) f -> b p w f", p=128, w=2)
        nblk = hp.shape[0]
        for b in range(nblk):
            ht = pool.tile([128, 2, DIM], F32, tag="ht")
            nc.sync.dma_start(ht[:], hp[b])
            nc.vector.tensor_tensor(
                out=ht[:], in0=ht[:],
                in1=scale[:, None, :].to_broadcast([128, 2, DIM]), op=mul)
            nc.vector.tensor_tensor(
                out=ht[:], in0=ht[:],
                in1=shift[:, None, :].to_broadcast([128, 2, DIM]), op=add)
            nc.scalar.dma_start(ho[b], ht[:])
    return _split_excess_waits(nc)


# ------------------------------------------------------------------- runner

def _install_ntff_hook():
    """Install the antenv.axon_hooks shim so run_bass_kernel_spmd(trace=True)
    can capture NTFF profiles through the axon .so."""
    import sys, types
    if "antenv.axon_hooks" in sys.modules:
        return True
    try:
        import antenv
        from trn_agent_boot.trn_boot import _ntff_profile_via_ctypes
        mod = types.ModuleType("antenv.axon_hooks")
        mod._hook = _ntff_profile_via_ctypes("/opt/axon/libaxon_pjrt.so")
        mod.set_axon_ntff_profile_hook = lambda h: setattr(mod, "_hook", h)
        mod.get_axon_ntff_profile_hook = lambda: mod._hook
        sys.modules["antenv.axon_hooks"] = mod
        antenv.axon_hooks = mod
        return mod._hook is not None
    except Exception:
        return False


_CACHE = {}


def _get_phase1(npc, win, tpw, xrows, prec):
    key = ("p1", npc, win, tpw, xrows, prec)
    if key not in _CACHE:
        _CACHE[key] = _build_phase1(npc, win, tpw, xrows, prec)
    return _CACHE[key]


def _get_phase2(win):
    key = ("p2", win)
    if key not in _CACHE:
        _CACHE[key] = _build_phase2(win)
    return _CACHE[key]


def run_pipeline(x, edge_attr, WQ, WK, WE, WV, gamma, beta, edge_index,
                 cores=CORES, npc=NPC, prec="f32r", timed=False):
    n = cores * npc
    prep = _prep(x, edge_attr, edge_index, cores, npc)
    tpw, win, xrows = prep["tpw"], prep["win"], prep["xrows"]
    scale_inv = np.float32(1.0 / math.sqrt(DH))

    wq = np.ascontiguousarray((np.asarray(WQ, np.float32) * scale_inv)
                              .reshape(2, 128, DIM))
    we = np.ascontiguousarray(np.asarray(WE, np.float32).reshape(2, 128, DIM))
    wkv = np.ascontiguousarray(np.concatenate(
        [np.asarray(WK, np.float32).reshape(2, 128, DIM),
         np.asarray(WV, np.float32).reshape(2, 128, DIM)], axis=2))

    if prec == "bf16":
        import ml_dtypes
        cast = lambda a: a.astype(ml_dtypes.bfloat16)
    else:
        cast = lambda a: np.ascontiguousarray(a, dtype=np.float32)
    nc1 = _get_phase1(npc, win, tpw, xrows, prec)
    xpad = prep["xpad"]
    xtab = cast(xpad)
    cc = prep["consts"]
    consts = dict(iota_rowb=cast(cc["iota_rowb"]), iota_col=cc["iota_col"],
                  ident=cast(cc["ident"]), ones_row=cast(cc["ones_row"]))
    in_maps = []
    for c in range(cores):
        pc = prep["per_core"][c]
        xwin_c = np.ascontiguousarray(xpad[c * npc:c * npc + win * 128])
        in_maps.append(dict(
            xtab=xtab, xwin=xwin_c, eat=cast(pc["eat"]), idxcol=pc["idxcol"],
            slotcol=pc["slotcol"], slotrow=cast(pc["slotrow"]),
            xwint=cast(pc["xwint"]),
            wq=cast(wq), we=cast(we), wkv=cast(wkv), **consts))

    t1 = None
    trace = timed and _install_ntff_hook()
    r1 = run_bass_kernel_spmd(nc1, in_maps, list(range(cores)), trace=trace)
    results1 = r1.results
    t1 = r1.exec_time_ns
    hpres = [results1[c]["hpre"] for c in range(cores)]
    bns = sum(results1[c]["bns"].astype(np.float64) for c in range(cores))
    mean = bns[0] / n
    var = bns[1] / n - mean * mean
    scale = (np.asarray(gamma, np.float64) / np.sqrt(var + EPS_BN))
    shift = np.asarray(beta, np.float64) - mean * scale

    nc2 = _get_phase2(win)
    in_maps2 = [dict(hpre=hpres[c],
                     scale_rep=np.ascontiguousarray(
                         np.tile(scale.astype(np.float32), (128, 1))),
                     shift_rep=np.ascontiguousarray(
                         np.tile(shift.astype(np.float32), (128, 1))))
                for c in range(cores)]
    r2 = run_bass_kernel_spmd(nc2, in_maps2, list(range(cores)), trace=trace)
    results2 = r2.results
    t2 = r2.exec_time_ns
    out = np.concatenate([results2[c]["hout"][:npc] for c in range(cores)])
    info = dict(t1=t1, t2=t2, tpw=tpw)
    return out.astype(np.float32), info


def kernel(x, edge_attr, WQ, WK, WE, WV, gamma, beta, edge_index):
    out, _ = run_pipeline(x, edge_attr, WQ, WK, WE, WV, gamma, beta, edge_index)
    return out
